# revision 1
# baseline (speedup 1.0000x reference)
"""CogVLM VisionExpert MLP (moe routing) on 8 trn2 NeuronCores.

Strategy:
  - Host computes the vision/language routing mask in numpy and permutes
    tokens by expert. Each token goes through exactly ONE expert (the
    reference computes both and selects; routing halves the matmul work).
  - Default sharding (expert-affinity DP4 x TP2): four 2-core tensor-parallel
    pairs, each pair owns one expert and a contiguous slice of that expert's
    tokens (for this problem's ~3:1 lang:vis split: 3 lang pairs + 1 vis
    pair). TP halves split the intermediate dim I=11008 -> 5504 = 43*128
    exactly, so the icol padding TP8 needs (1376->1408, +2.3% matmuls)
    vanishes, and every token block is >=~512 wide so the per-block weight
    stream hides under the matmuls. Host sums each pair's two partial
    outputs (the "all-reduce") and un-permutes.
  - Fallback (TP8 over I, all tokens on every core) when the expert split is
    too skewed for affinity pairs to balance.
  - bf16 matmuls with fp32 PSUM accumulation; no token padding anywhere (the
    matmul moving dim accepts any width <= 512; ragged blocks use balanced
    sub-widths so no tiny trailing matmuls).

Layouts shipped to the device (everything lands in SBUF with natural
[128-partition, free] shape and fully-contiguous DMA runs):
  xt   [NK, 128, T]       bf16   xt[k, p, t]       = X_perm[t, k*128+p]
  g/u  [ni, 128, NK*128]  bf16   g[it, p, k*128+c] = Wg_sh[k*128+p, it*128+c]
  d    [NK, 128, ni*128]  bf16   d[h, p, i*128+c]  = Wd_sh[i*128+p, h*128+c]
  yt   [NK, 128, T]       f32    yt[h, p, t]       = Y_part[t, h*128+p]
"""
import os
import numpy as np
import ml_dtypes

import concourse.bacc as bacc
import concourse.mybir as mybir
import concourse.tile as tile
from concourse.bass_utils import run_bass_kernel_spmd
from concourse.bass_interp import get_hw_module

bf16 = ml_dtypes.bfloat16
B, S, H, I = 2, 2048, 4096, 11008
NCORES = 8
NK = H // 128                # 32 h tiles
TSUB = 512                   # matmul moving-dim chunk (one PSUM bank of f32)
VISION_TOKEN_TYPE = 1

# TP8 fallback geometry
ISH8 = I // 8                # 1376 real icols per core
NI8 = (ISH8 + 127) // 128    # 11 tiles (padded to 1408)
# expert-affinity DP4 x TP2 geometry
ISH2 = I // 2                # 5504 icols per core
NI2 = ISH2 // 128            # 43 tiles, exact

FP32 = mybir.dt.float32
BF16 = mybir.dt.bfloat16

_nc_cache = {}

# observability for test harnesses (not used by grading)
last_results = None
last_run = None  # (nc, in_maps) of the most recent kernel() call


def _plan_blocks(n, tblk):
    """Split a token count into blocks of <= tblk+128 (weights re-stream once
    per block). A remainder <=128 is folded into the last block: a standalone
    narrow block starves the PE on its weight stream."""
    out = []
    t = 0
    while n - t >= tblk:
        out.append((t, tblk))
        t += tblk
    r = n - t
    if r > 0:
        if r <= 128 and out:
            t0, tc = out[-1]
            out[-1] = (t0, tc + r)
        else:
            out.append((t, r))
    return out


def _subs(tcols):
    """Split a block into balanced matmul moving-dim pieces (<= 512 each),
    e.g. 1057 -> 353/352/352 — avoids tiny trailing matmuls whose LDWEIGHTS
    cost can't hide under the streaming."""
    nsub = (tcols + TSUB - 1) // TSUB
    base, rem = divmod(tcols, nsub)
    out = []
    c = 0
    for s in range(nsub):
        w = base + (1 if s < rem else 0)
        out.append((c, w))
        c += w
    return out


def _build(Tt, ni, weight_sets, blocks, a_bufs, wd_bufs, y_bufs):
    """Emit the SPMD program: for each (set_key, t0, tcols) block run the
    swiglu MLP over that token window with that weight set's shards."""
    nc = bacc.Bacc("TRN2", target_bir_lowering=False, debug=False,
                   num_devices=NCORES)

    xt_d = nc.dram_tensor("xt", [NK, 128, Tt], BF16, kind="ExternalInput")
    w_d = {}
    for key, (gn, un, dn) in weight_sets.items():
        w_d[key] = (
            nc.dram_tensor(gn, [ni, 128, NK * 128], BF16, kind="ExternalInput"),
            nc.dram_tensor(un, [ni, 128, NK * 128], BF16, kind="ExternalInput"),
            nc.dram_tensor(dn, [NK, 128, ni * 128], BF16, kind="ExternalInput"),
        )
    yt_d = nc.dram_tensor("yt", [NK, 128, Tt], FP32, kind="ExternalOutput")

    with tile.TileContext(nc) as tc_:
        with (
            tc_.tile_pool(name="xp", bufs=1) as xp,
            tc_.tile_pool(name="apool", bufs=a_bufs) as apool,
            tc_.tile_pool(name="wgu", bufs=2) as wgu,
            tc_.tile_pool(name="wdp", bufs=wd_bufs) as wdp,
            tc_.tile_pool(name="sp", bufs=2) as sp,
            tc_.tile_pool(name="yp", bufs=y_bufs) as yp,
            tc_.tile_pool(name="pg", bufs=2, space="PSUM") as pgp,
            tc_.tile_pool(name="pu", bufs=2, space="PSUM") as pup,
            tc_.tile_pool(name="py", bufs=4, space="PSUM") as pyp,
        ):
            for bi, (key, t0, tcols) in enumerate(blocks):
                g_d, u_d, d_d = w_d[key]
                subs = _subs(tcols)
                # issue the first gate/up weight DMAs before the x block so
                # the opening matmuls aren't queued behind the x traffic
                wg0 = wgu.tile([128, NK * 128], BF16, tag="wg", name="wg0")
                wu0 = wgu.tile([128, NK * 128], BF16, tag="wu", name="wu0")
                # per-k x tiles: fine-grained deps let the first matmuls start
                # as soon as their own h-slice lands, not the whole block
                x_sb = [xp.tile([128, tcols], BF16, tag=f"x{k}", name=f"xsb{k}")
                        for k in range(NK)]
                wgu1 = None
                if bi == 0:
                    # kernel startup is latency-critical: land x[0] first,
                    # spread the first weight tiles over 4 DMA queues each,
                    # and interleave it=1's weight chunks into the x stream
                    # so they arrive just in time for the second icol pass
                    nc.sync.dma_start(x_sb[0][:], xt_d.ap()[0, :, t0:t0 + tcols])
                    q4 = NK * 128 // 4
                    for q in range(4):
                        nc.sync.dma_start(wg0[:, q * q4:(q + 1) * q4],
                                          g_d.ap()[0, :, q * q4:(q + 1) * q4])
                    for q in range(4):
                        nc.sync.dma_start(wu0[:, q * q4:(q + 1) * q4],
                                          u_d.ap()[0, :, q * q4:(q + 1) * q4])
                    chunks = []
                    if ni > 1:
                        wg1 = wgu.tile([128, NK * 128], BF16, tag="wg", name="wg1")
                        wu1 = wgu.tile([128, NK * 128], BF16, tag="wu", name="wu1")
                        wgu1 = (wg1, wu1)
                        chunks = ([(wg1, g_d, q) for q in range(4)]
                                  + [(wu1, u_d, q) for q in range(4)])
                    ci = 0
                    for k in range(1, NK):
                        nc.sync.dma_start(x_sb[k][:], xt_d.ap()[k, :, t0:t0 + tcols])
                        if (k % 4 == 0 or k == NK - 1) and ci < len(chunks):
                            t_, d_, q = chunks[ci]
                            ci += 1
                            nc.sync.dma_start(t_[:, q * q4:(q + 1) * q4],
                                              d_.ap()[1, :, q * q4:(q + 1) * q4])
                    while ci < len(chunks):
                        t_, d_, q = chunks[ci]
                        ci += 1
                        nc.sync.dma_start(t_[:, q * q4:(q + 1) * q4],
                                          d_.ap()[1, :, q * q4:(q + 1) * q4])
                else:
                    nc.sync.dma_start(wg0[:], g_d.ap()[0])
                    nc.sync.dma_start(wu0[:], u_d.ap()[0])
                    for k in range(NK):
                        nc.sync.dma_start(x_sb[k][:], xt_d.ap()[k, :, t0:t0 + tcols])
                a_sb = apool.tile([128, ni, tcols], BF16, tag="a")
                for it in range(ni):
                    if it == 0:
                        wg_sb, wu_sb = wg0, wu0
                    elif it == 1 and wgu1 is not None:
                        wg_sb, wu_sb = wgu1
                    else:
                        wg_sb = wgu.tile([128, NK * 128], BF16, tag="wg")
                        wu_sb = wgu.tile([128, NK * 128], BF16, tag="wu")
                        nc.sync.dma_start(wg_sb[:], g_d.ap()[it])
                        nc.sync.dma_start(wu_sb[:], u_d.ap()[it])
                    for (c0, w) in subs:
                        c1 = c0 + w
                        pg = pgp.tile([128, w], FP32, tag="pg")
                        pu = pup.tile([128, w], FP32, tag="pu")
                        for k in range(NK):
                            nc.tensor.matmul(pg[:], wg_sb[:, k * 128:(k + 1) * 128],
                                             x_sb[k][:, c0:c1],
                                             start=(k == 0), stop=(k == NK - 1))
                        for k in range(NK):
                            nc.tensor.matmul(pu[:], wu_sb[:, k * 128:(k + 1) * 128],
                                             x_sb[k][:, c0:c1],
                                             start=(k == 0), stop=(k == NK - 1))
                        silu_sb = sp.tile([128, w], FP32, tag="silu")
                        nc.scalar.activation(silu_sb[:], pg[:],
                                             mybir.ActivationFunctionType.Silu)
                        nc.vector.tensor_mul(a_sb[:, it, c0:c1], silu_sb[:], pu[:])
                for h in range(NK):
                    wd_sb = wdp.tile([128, ni * 128], BF16, tag="wd")
                    nc.sync.dma_start(wd_sb[:], d_d.ap()[h])
                    for (c0, w) in subs:
                        c1 = c0 + w
                        py = pyp.tile([128, w], FP32, tag="py")
                        for i in range(ni):
                            nc.tensor.matmul(py[:], wd_sb[:, i * 128:(i + 1) * 128],
                                             a_sb[:, i, c0:c1],
                                             start=(i == 0), stop=(i == ni - 1))
                        y_sb = yp.tile([128, w], FP32, tag="y")
                        nc.scalar.copy(y_sb[:], py[:])
                        nc.sync.dma_start(yt_d.ap()[h, :, t0 + c0:t0 + c1], y_sb[:])

    nc.compile()
    nc.m = get_hw_module(nc.m)
    return nc


def _build_tp8(nl, nv):
    blocks = [("l", t0, tc) for (t0, tc) in _plan_blocks(nl, 1024)]
    blocks += [("v", nl + t0, tc) for (t0, tc) in _plan_blocks(nv, 1024)]
    return _build(nl + nv, NI8,
                  {"l": ("gl", "ul", "dl"), "v": ("gv", "uv", "dv")},
                  blocks, a_bufs=2, wd_bufs=3, y_bufs=4)


def _build_aff(cap):
    # blocks of ~512 keep the [128, 43, tcols] "a" tile within SBUF
    blocks = [("e", t0, tc) for (t0, tc) in _plan_blocks(cap, 512)]
    return _build(cap, NI2, {"e": ("g", "u", "d")},
                  blocks, a_bufs=2, wd_bufs=2, y_bufs=2)


def _tile_gu(W, c, ish, ni):
    """[H, I] f32 -> per-core [ni, 128, NK*128] bf16 column shard."""
    sh = np.asarray(W, dtype=np.float32)[:, c * ish:(c + 1) * ish].astype(bf16)
    pad = ni * 128 - ish
    if pad:
        sh = np.concatenate([sh, np.zeros((H, pad), dtype=bf16)], axis=1)
    t = sh.reshape(NK, 128, ni, 128).transpose(2, 1, 0, 3)
    return np.ascontiguousarray(t).reshape(ni, 128, NK * 128)


def _tile_d(W, c, ish, ni):
    """[I, H] f32 -> per-core [NK, 128, ni*128] bf16 row shard."""
    sh = np.asarray(W, dtype=np.float32)[c * ish:(c + 1) * ish, :].astype(bf16)
    pad = ni * 128 - ish
    if pad:
        sh = np.concatenate([sh, np.zeros((pad, H), dtype=bf16)], axis=0)
    t = sh.reshape(ni, 128, NK, 128).transpose(2, 1, 0, 3)
    return np.ascontiguousarray(t).reshape(NK, 128, ni * 128)


def _chunks(n, k):
    if k <= 0:
        return []
    base, rem = divmod(n, k)
    out, s = [], 0
    for i in range(k):
        c = base + (1 if i < rem else 0)
        out.append((s, c))
        s += c
    return out


def _affinity_shards(Nl, Nv):
    """4 single-expert token shards for the DP4 x TP2 layout, or None if the
    expert split is too skewed for this to beat TP8."""
    if Nl == 0 or Nv == 0:
        k_l = 4 if Nv == 0 else 0
    else:
        k_l = min(3, max(1, round(4 * Nl / (Nl + Nv))))
    shards = ([("l", s, c) for (s, c) in _chunks(Nl, k_l)]
              + [("v", s, c) for (s, c) in _chunks(Nv, 4 - k_l)])
    if len(shards) != 4 or any(c == 0 for _, _, c in shards):
        return None, 0
    cap = max(c for _, _, c in shards)
    # affinity wins only while its per-core work (cap x 5504 exact icols)
    # undercuts TP8's (all tokens x 1408 padded icols)
    if cap * ISH2 >= (Nl + Nv) * NI8 * 128:
        return None, 0
    return shards, cap


def kernel(hidden_states, token_type_ids, lang_gate, lang_up, lang_down,
           vis_gate, vis_up, vis_down):
    global last_results, last_run
    x = np.asarray(hidden_states, dtype=np.float32).reshape(B * S, H)
    tt = np.asarray(token_type_ids).reshape(B, S)

    vis = np.zeros((B, S), dtype=bool)
    vis[:, :-1] = (tt[:, :-1] == VISION_TOKEN_TYPE) & (tt[:, 1:] == VISION_TOKEN_TYPE)
    visf = vis.reshape(-1)
    lang_idx = np.flatnonzero(~visf)
    vis_idx = np.flatnonzero(visf)
    Nl, Nv = len(lang_idx), len(vis_idx)
    ew = {"l": (lang_gate, lang_up, lang_down), "v": (vis_gate, vis_up, vis_down)}

    shards, cap = _affinity_shards(Nl, Nv)
    if shards is not None:
        # ---- expert-affinity DP4 x TP2 ----
        key = ("aff", cap)
        if key not in _nc_cache:
            _nc_cache[key] = _build_aff(cap)
        nc = _nc_cache[key]

        wt = {}  # (expert, tp) -> tiled weights
        for e in set(e for e, _, _ in shards):
            g, u, d = ew[e]
            for tp in range(2):
                wt[(e, tp)] = (_tile_gu(g, tp, ISH2, NI2),
                               _tile_gu(u, tp, ISH2, NI2),
                               _tile_d(d, tp, ISH2, NI2))
        in_maps = [None] * NCORES
        shard_idx = []
        for s, (e, st, cnt) in enumerate(shards):
            idx = (lang_idx if e == "l" else vis_idx)[st:st + cnt]
            shard_idx.append(idx)
            xs = np.zeros((cap, H), dtype=np.float32)
            xs[:cnt] = x[idx]
            xt_s = np.ascontiguousarray(xs.T.astype(bf16)).reshape(NK, 128, cap)
            for tp in range(2):
                g_t, u_t, d_t = wt[(e, tp)]
                in_maps[2 * s + tp] = {"xt": xt_s, "g": g_t, "u": u_t, "d": d_t}

        trace = bool(int(os.environ.get("KERNEL_TRACE", "0")))
        res = run_bass_kernel_spmd(nc, in_maps, list(range(NCORES)), trace=trace)
        last_results = res
        last_run = (nc, in_maps)

        out_flat = np.empty((B * S, H), dtype=np.float32)
        for s, (e, st, cnt) in enumerate(shards):
            ysum = (res.results[2 * s]["yt"] + res.results[2 * s + 1]["yt"])
            out_flat[shard_idx[s]] = ysum.reshape(H, cap)[:, :cnt].T
        return out_flat.reshape(B, S, H)

    # ---- TP8 fallback: shard I 8 ways, every core runs all tokens ----
    Tt = Nl + Nv
    xp_ = np.empty((Tt, H), dtype=np.float32)
    xp_[:Nl] = x[lang_idx]
    xp_[Nl:] = x[vis_idx]
    xt = np.ascontiguousarray(xp_.T.astype(bf16)).reshape(NK, 128, Tt)

    key = ("tp8", Nl, Nv)
    if key not in _nc_cache:
        _nc_cache[key] = _build_tp8(Nl, Nv)
    nc = _nc_cache[key]

    in_maps = []
    for c in range(NCORES):
        in_maps.append({
            "xt": xt,
            "gl": _tile_gu(lang_gate, c, ISH8, NI8),
            "ul": _tile_gu(lang_up, c, ISH8, NI8),
            "dl": _tile_d(lang_down, c, ISH8, NI8),
            "gv": _tile_gu(vis_gate, c, ISH8, NI8),
            "uv": _tile_gu(vis_up, c, ISH8, NI8),
            "dv": _tile_d(vis_down, c, ISH8, NI8),
        })

    trace = bool(int(os.environ.get("KERNEL_TRACE", "0")))
    res = run_bass_kernel_spmd(nc, in_maps, list(range(NCORES)), trace=trace)
    last_results = res
    last_run = (nc, in_maps)

    ysum = np.zeros((NK, 128, Tt), dtype=np.float32)
    for r in res.results:
        ysum += r["yt"]
    yt_full = ysum.reshape(H, Tt)
    out_flat = np.empty((B * S, H), dtype=np.float32)
    out_flat[lang_idx] = yt_full[:, :Nl].T
    out_flat[vis_idx] = yt_full[:, Nl:].T
    return out_flat.reshape(B, S, H)



# revision 5
# speedup vs baseline: 1.7278x; 1.7278x over previous
"""CogVLM VisionExpert MLP (moe routing) on 8 trn2 NeuronCores.

Strategy:
  - Host computes the vision/language routing mask in numpy and permutes
    tokens by expert. Each token goes through exactly ONE expert (the
    reference computes both and selects; routing halves the matmul work).
  - Default sharding (expert-affinity DP4 x TP2): four 2-core tensor-parallel
    pairs, each pair owns one expert and a contiguous slice of that expert's
    tokens (for this problem's ~3:1 lang:vis split: 3 lang pairs + 1 vis
    pair). TP halves split the intermediate dim I=11008 -> 5504 = 43*128
    exactly, so the icol padding TP8 needs (1376->1408, +2.3% matmuls)
    vanishes, and every token block is >=~512 wide so the per-block weight
    stream hides under the matmuls. Host sums each pair's two partial
    outputs (the "all-reduce") and un-permutes.
  - Fallback (TP8 over I, all tokens on every core) when the expert split is
    too skewed for affinity pairs to balance.
  - bf16 matmuls with fp32 PSUM accumulation; no token padding anywhere (the
    matmul moving dim accepts any width <= 512; ragged blocks use balanced
    sub-widths so no tiny trailing matmuls).

Layouts shipped to the device (everything lands in SBUF with natural
[128-partition, free] shape and fully-contiguous DMA runs):
  xt   [NK, 128, T]       bf16   xt[k, p, t]       = X_perm[t, k*128+p]
  g/u  [ni, 128, NK*128]  bf16   g[it, p, k*128+c] = Wg_sh[k*128+p, it*128+c]
  d    [NK, 128, ni*128]  bf16   d[h, p, i*128+c]  = Wd_sh[i*128+p, h*128+c]
  yt   [NK, 128, T]       f32    yt[h, p, t]       = Y_part[t, h*128+p]
"""
import os
import numpy as np
import ml_dtypes

import concourse.bacc as bacc
import concourse.mybir as mybir
import concourse.tile as tile
from concourse.bass_utils import run_bass_kernel_spmd
from concourse.bass_interp import get_hw_module

bf16 = ml_dtypes.bfloat16
B, S, H, I = 2, 2048, 4096, 11008
NCORES = 8
NK = H // 128                # 32 h tiles
TSUB = 512                   # matmul moving-dim chunk (one PSUM bank of f32)
VISION_TOKEN_TYPE = 1

# TP8 fallback geometry
ISH8 = I // 8                # 1376 real icols per core
NI8 = (ISH8 + 127) // 128    # 11 tiles (padded to 1408)
# expert-affinity DP4 x TP2 geometry
ISH2 = I // 2                # 5504 icols per core
NI2 = ISH2 // 128            # 43 tiles, exact

FP32 = mybir.dt.float32
BF16 = mybir.dt.bfloat16

_nc_cache = {}

# observability for test harnesses (not used by grading)
last_results = None
last_run = None  # (nc, in_maps) of the most recent kernel() call


def _plan_blocks(n, tblk):
    """Split a token count into blocks of <= tblk+128 (weights re-stream once
    per block). A remainder <=128 is folded into the last block: a standalone
    narrow block starves the PE on its weight stream."""
    out = []
    t = 0
    while n - t >= tblk:
        out.append((t, tblk))
        t += tblk
    r = n - t
    if r > 0:
        if r <= 128 and out:
            t0, tc = out[-1]
            out[-1] = (t0, tc + r)
        else:
            out.append((t, r))
    return out


def _subs(tcols):
    """Split a block into balanced matmul moving-dim pieces (<= 512 each),
    e.g. 1057 -> 353/352/352 — avoids tiny trailing matmuls whose LDWEIGHTS
    cost can't hide under the streaming."""
    nsub = (tcols + TSUB - 1) // TSUB
    base, rem = divmod(tcols, nsub)
    out = []
    c = 0
    for s in range(nsub):
        w = base + (1 if s < rem else 0)
        out.append((c, w))
        c += w
    return out


def _build(Tt, ni, weight_sets, blocks, a_bufs, wd_bufs, y_bufs):
    """Emit the SPMD program: for each (set_key, t0, tcols) block run the
    swiglu MLP over that token window with that weight set's shards."""
    nc = bacc.Bacc("TRN2", target_bir_lowering=False, debug=False,
                   num_devices=NCORES)

    xt_d = nc.dram_tensor("xt", [NK, 128, Tt], BF16, kind="ExternalInput")
    w_d = {}
    for key, (gn, un, dn) in weight_sets.items():
        w_d[key] = (
            nc.dram_tensor(gn, [ni, 128, NK * 128], BF16, kind="ExternalInput"),
            nc.dram_tensor(un, [ni, 128, NK * 128], BF16, kind="ExternalInput"),
            nc.dram_tensor(dn, [NK, 128, ni * 128], BF16, kind="ExternalInput"),
        )
    yt_d = nc.dram_tensor("yt", [NK, 128, Tt], FP32, kind="ExternalOutput")

    with tile.TileContext(nc) as tc_:
        with (
            tc_.tile_pool(name="xp", bufs=1) as xp,
            tc_.tile_pool(name="apool", bufs=a_bufs) as apool,
            tc_.tile_pool(name="wgu", bufs=2) as wgu,
            tc_.tile_pool(name="wdp", bufs=wd_bufs) as wdp,
            tc_.tile_pool(name="sp", bufs=2) as sp,
            tc_.tile_pool(name="yp", bufs=y_bufs) as yp,
            tc_.tile_pool(name="pg", bufs=3, space="PSUM") as pgp,
            tc_.tile_pool(name="pu", bufs=3, space="PSUM") as pup,
            tc_.tile_pool(name="py", bufs=2, space="PSUM") as pyp,
        ):
            for bi, (key, t0, tcols) in enumerate(blocks):
                g_d, u_d, d_d = w_d[key]
                subs = _subs(tcols)
                # issue the first gate/up weight DMAs before the x block so
                # the opening matmuls aren't queued behind the x traffic
                wg0 = wgu.tile([128, NK * 128], BF16, tag="wg", name="wg0")
                wu0 = wgu.tile([128, NK * 128], BF16, tag="wu", name="wu0")
                # per-k x tiles: fine-grained deps let the first matmuls start
                # as soon as their own h-slice lands, not the whole block
                x_sb = [xp.tile([128, tcols], BF16, tag=f"x{k}", name=f"xsb{k}")
                        for k in range(NK)]
                wgu1 = None
                if bi == 0:
                    # kernel startup is latency-critical: land x[0] first,
                    # spread the first weight tiles over 4 DMA queues each,
                    # and interleave it=1's weight chunks into the x stream
                    # so they arrive just in time for the second icol pass
                    nc.sync.dma_start(x_sb[0][:], xt_d.ap()[0, :, t0:t0 + tcols])
                    q4 = NK * 128 // 4
                    for q in range(4):
                        nc.sync.dma_start(wg0[:, q * q4:(q + 1) * q4],
                                          g_d.ap()[0, :, q * q4:(q + 1) * q4])
                    for q in range(4):
                        nc.sync.dma_start(wu0[:, q * q4:(q + 1) * q4],
                                          u_d.ap()[0, :, q * q4:(q + 1) * q4])
                    chunks = []
                    if ni > 1:
                        wg1 = wgu.tile([128, NK * 128], BF16, tag="wg", name="wg1")
                        wu1 = wgu.tile([128, NK * 128], BF16, tag="wu", name="wu1")
                        wgu1 = (wg1, wu1)
                        chunks = ([(wg1, g_d, q) for q in range(4)]
                                  + [(wu1, u_d, q) for q in range(4)])
                    ci = 0
                    for k in range(1, NK):
                        nc.sync.dma_start(x_sb[k][:], xt_d.ap()[k, :, t0:t0 + tcols])
                        if (k % 4 == 0 or k == NK - 1) and ci < len(chunks):
                            t_, d_, q = chunks[ci]
                            ci += 1
                            nc.sync.dma_start(t_[:, q * q4:(q + 1) * q4],
                                              d_.ap()[1, :, q * q4:(q + 1) * q4])
                    while ci < len(chunks):
                        t_, d_, q = chunks[ci]
                        ci += 1
                        nc.sync.dma_start(t_[:, q * q4:(q + 1) * q4],
                                          d_.ap()[1, :, q * q4:(q + 1) * q4])
                else:
                    h2 = NK * 128 // 2
                    for q in range(2):
                        nc.sync.dma_start(wg0[:, q * h2:(q + 1) * h2],
                                          g_d.ap()[0, :, q * h2:(q + 1) * h2])
                        nc.sync.dma_start(wu0[:, q * h2:(q + 1) * h2],
                                          u_d.ap()[0, :, q * h2:(q + 1) * h2])
                    for k in range(NK):
                        nc.sync.dma_start(x_sb[k][:], xt_d.ap()[k, :, t0:t0 + tcols])
                a_sb = apool.tile([128, ni, tcols], BF16, tag="a")
                for it in range(ni):
                    if it == 0:
                        wg_sb, wu_sb = wg0, wu0
                    elif it == 1 and wgu1 is not None:
                        wg_sb, wu_sb = wgu1
                    else:
                        wg_sb = wgu.tile([128, NK * 128], BF16, tag="wg")
                        wu_sb = wgu.tile([128, NK * 128], BF16, tag="wu")
                        h2 = NK * 128 // 2
                        for q in range(2):
                            nc.sync.dma_start(wg_sb[:, q * h2:(q + 1) * h2],
                                              g_d.ap()[it, :, q * h2:(q + 1) * h2])
                            nc.sync.dma_start(wu_sb[:, q * h2:(q + 1) * h2],
                                              u_d.ap()[it, :, q * h2:(q + 1) * h2])
                    for (c0, w) in subs:
                        c1 = c0 + w
                        pg = pgp.tile([128, w], FP32, tag="pg")
                        pu = pup.tile([128, w], FP32, tag="pu")
                        for k in range(NK):
                            nc.tensor.matmul(pg[:], wg_sb[:, k * 128:(k + 1) * 128],
                                             x_sb[k][:, c0:c1],
                                             start=(k == 0), stop=(k == NK - 1))
                        for k in range(NK):
                            nc.tensor.matmul(pu[:], wu_sb[:, k * 128:(k + 1) * 128],
                                             x_sb[k][:, c0:c1],
                                             start=(k == 0), stop=(k == NK - 1))
                        silu_sb = sp.tile([128, w], FP32, tag="silu")
                        nc.scalar.activation(silu_sb[:], pg[:],
                                             mybir.ActivationFunctionType.Silu)
                        nc.vector.tensor_mul(a_sb[:, it, c0:c1], silu_sb[:], pu[:])
                for h in range(NK):
                    wd_sb = wdp.tile([128, ni * 128], BF16, tag="wd")
                    i2 = ni * 128 // 2
                    for q in range(2):
                        nc.sync.dma_start(wd_sb[:, q * i2:(q + 1) * i2],
                                          d_d.ap()[h, :, q * i2:(q + 1) * i2])
                    for (c0, w) in subs:
                        c1 = c0 + w
                        py = pyp.tile([128, w], FP32, tag="py")
                        for i in range(ni):
                            nc.tensor.matmul(py[:], wd_sb[:, i * 128:(i + 1) * 128],
                                             a_sb[:, i, c0:c1],
                                             start=(i == 0), stop=(i == ni - 1))
                        y_sb = yp.tile([128, w], FP32, tag="y")
                        nc.scalar.copy(y_sb[:], py[:])
                        nc.sync.dma_start(yt_d.ap()[h, :, t0 + c0:t0 + c1], y_sb[:])

    nc.compile()
    nc.m = get_hw_module(nc.m)
    return nc


def _build_tp8(nl, nv):
    blocks = [("l", t0, tc) for (t0, tc) in _plan_blocks(nl, 1024)]
    blocks += [("v", nl + t0, tc) for (t0, tc) in _plan_blocks(nv, 1024)]
    return _build(nl + nv, NI8,
                  {"l": ("gl", "ul", "dl"), "v": ("gv", "uv", "dv")},
                  blocks, a_bufs=2, wd_bufs=3, y_bufs=4)


def _build_aff(cap):
    # blocks of ~512 keep the [128, 43, tcols] "a" tile within SBUF
    blocks = [("e", t0, tc) for (t0, tc) in _plan_blocks(cap, 512)]
    return _build(cap, NI2, {"e": ("g", "u", "d")},
                  blocks, a_bufs=2, wd_bufs=2, y_bufs=2)


def _tile_gu(W, c, ish, ni):
    """[H, I] f32 -> per-core [ni, 128, NK*128] bf16 column shard."""
    sh = np.asarray(W, dtype=np.float32)[:, c * ish:(c + 1) * ish].astype(bf16)
    pad = ni * 128 - ish
    if pad:
        sh = np.concatenate([sh, np.zeros((H, pad), dtype=bf16)], axis=1)
    t = sh.reshape(NK, 128, ni, 128).transpose(2, 1, 0, 3)
    return np.ascontiguousarray(t).reshape(ni, 128, NK * 128)


def _tile_d(W, c, ish, ni):
    """[I, H] f32 -> per-core [NK, 128, ni*128] bf16 row shard."""
    sh = np.asarray(W, dtype=np.float32)[c * ish:(c + 1) * ish, :].astype(bf16)
    pad = ni * 128 - ish
    if pad:
        sh = np.concatenate([sh, np.zeros((pad, H), dtype=bf16)], axis=0)
    t = sh.reshape(ni, 128, NK, 128).transpose(2, 1, 0, 3)
    return np.ascontiguousarray(t).reshape(NK, 128, ni * 128)


def _chunks(n, k):
    if k <= 0:
        return []
    base, rem = divmod(n, k)
    out, s = [], 0
    for i in range(k):
        c = base + (1 if i < rem else 0)
        out.append((s, c))
        s += c
    return out


def _affinity_shards(Nl, Nv):
    """4 single-expert token shards for the DP4 x TP2 layout, or None if the
    expert split is too skewed for this to beat TP8."""
    if Nl == 0 or Nv == 0:
        k_l = 4 if Nv == 0 else 0
    else:
        k_l = min(3, max(1, round(4 * Nl / (Nl + Nv))))
    shards = ([("l", s, c) for (s, c) in _chunks(Nl, k_l)]
              + [("v", s, c) for (s, c) in _chunks(Nv, 4 - k_l)])
    if len(shards) != 4 or any(c == 0 for _, _, c in shards):
        return None, 0
    cap = max(c for _, _, c in shards)
    # affinity wins only while its per-core work (cap x 5504 exact icols)
    # undercuts TP8's (all tokens x 1408 padded icols)
    if cap * ISH2 >= (Nl + Nv) * NI8 * 128:
        return None, 0
    return shards, cap


def kernel(hidden_states, token_type_ids, lang_gate, lang_up, lang_down,
           vis_gate, vis_up, vis_down):
    global last_results, last_run
    x = np.asarray(hidden_states, dtype=np.float32).reshape(B * S, H)
    tt = np.asarray(token_type_ids).reshape(B, S)

    vis = np.zeros((B, S), dtype=bool)
    vis[:, :-1] = (tt[:, :-1] == VISION_TOKEN_TYPE) & (tt[:, 1:] == VISION_TOKEN_TYPE)
    visf = vis.reshape(-1)
    lang_idx = np.flatnonzero(~visf)
    vis_idx = np.flatnonzero(visf)
    Nl, Nv = len(lang_idx), len(vis_idx)
    ew = {"l": (lang_gate, lang_up, lang_down), "v": (vis_gate, vis_up, vis_down)}

    shards, cap = _affinity_shards(Nl, Nv)
    if shards is not None:
        # ---- expert-affinity DP4 x TP2 ----
        key = ("aff", cap)
        if key not in _nc_cache:
            _nc_cache[key] = _build_aff(cap)
        nc = _nc_cache[key]

        wt = {}  # (expert, tp) -> tiled weights
        for e in set(e for e, _, _ in shards):
            g, u, d = ew[e]
            for tp in range(2):
                wt[(e, tp)] = (_tile_gu(g, tp, ISH2, NI2),
                               _tile_gu(u, tp, ISH2, NI2),
                               _tile_d(d, tp, ISH2, NI2))
        in_maps = [None] * NCORES
        shard_idx = []
        for s, (e, st, cnt) in enumerate(shards):
            idx = (lang_idx if e == "l" else vis_idx)[st:st + cnt]
            shard_idx.append(idx)
            xs = np.zeros((cap, H), dtype=np.float32)
            xs[:cnt] = x[idx]
            xt_s = np.ascontiguousarray(xs.T.astype(bf16)).reshape(NK, 128, cap)
            for tp in range(2):
                g_t, u_t, d_t = wt[(e, tp)]
                in_maps[2 * s + tp] = {"xt": xt_s, "g": g_t, "u": u_t, "d": d_t}

        trace = bool(int(os.environ.get("KERNEL_TRACE", "0")))
        res = run_bass_kernel_spmd(nc, in_maps, list(range(NCORES)), trace=trace)
        last_results = res
        last_run = (nc, in_maps)

        out_flat = np.empty((B * S, H), dtype=np.float32)
        for s, (e, st, cnt) in enumerate(shards):
            ysum = (res.results[2 * s]["yt"] + res.results[2 * s + 1]["yt"])
            out_flat[shard_idx[s]] = ysum.reshape(H, cap)[:, :cnt].T
        return out_flat.reshape(B, S, H)

    # ---- TP8 fallback: shard I 8 ways, every core runs all tokens ----
    Tt = Nl + Nv
    xp_ = np.empty((Tt, H), dtype=np.float32)
    xp_[:Nl] = x[lang_idx]
    xp_[Nl:] = x[vis_idx]
    xt = np.ascontiguousarray(xp_.T.astype(bf16)).reshape(NK, 128, Tt)

    key = ("tp8", Nl, Nv)
    if key not in _nc_cache:
        _nc_cache[key] = _build_tp8(Nl, Nv)
    nc = _nc_cache[key]

    in_maps = []
    for c in range(NCORES):
        in_maps.append({
            "xt": xt,
            "gl": _tile_gu(lang_gate, c, ISH8, NI8),
            "ul": _tile_gu(lang_up, c, ISH8, NI8),
            "dl": _tile_d(lang_down, c, ISH8, NI8),
            "gv": _tile_gu(vis_gate, c, ISH8, NI8),
            "uv": _tile_gu(vis_up, c, ISH8, NI8),
            "dv": _tile_d(vis_down, c, ISH8, NI8),
        })

    trace = bool(int(os.environ.get("KERNEL_TRACE", "0")))
    res = run_bass_kernel_spmd(nc, in_maps, list(range(NCORES)), trace=trace)
    last_results = res
    last_run = (nc, in_maps)

    ysum = np.zeros((NK, 128, Tt), dtype=np.float32)
    for r in res.results:
        ysum += r["yt"]
    yt_full = ysum.reshape(H, Tt)
    out_flat = np.empty((B * S, H), dtype=np.float32)
    out_flat[lang_idx] = yt_full[:, :Nl].T
    out_flat[vis_idx] = yt_full[:, Nl:].T
    return out_flat.reshape(B, S, H)

